# revision 80
# baseline (speedup 1.0000x reference)
"""MiniMax-Text-01 lightning attention layer on 8 Trainium2 NeuronCores (v2).

Sharding: core c = 4*b + g runs attention for batch b, heads [8g, 8g+8).
Phase 4 (RMSNorm/gate/out-proj) is sharded over 1024 INTERLEAVED tokens from
BOTH batches (half-blocks {8m + c : m=0..3} of 128 tokens in each batch), so
the 8-rank AllToAll carries no duplicated data: chunk m (blocks 4m..4m+3)
becomes ready at 25/50/75/100% of the attention scan and is exchanged
immediately, hiding the collective under compute.

Host pre-packs every input to fp16 in the exact SBUF layout (including the
pre-transposed hidden states), so the device runs almost pure matmul streams:
  phase 0: gt = sigmoid(w_gate.T @ hidT_q) -> DRAM fp16, while w_qk/w_v
           prefetch streams underneath on another DMA queue.
  phase 1: 8 block-pairs; per pair: v_sd / qT / kT projections (fp16, FWL)
           and 16 head-blocks of block-recurrent attention.  The per-head KV
           state for odd heads lives at partitions 64-127 via tile_position
           col-offset matmuls (no SBUF shift DMAs).  After pair 2m+1: A2A
           chunk m + assembly DMA into SBUF (gpsimd queue).
  tail:    per chunk: sumsq (ones-matmul) -> rstd; attnT *= normw*gate;
           y = (attnT).T @ w_out * rstd -> DRAM f32 (w_out streamed nn-major).

Measured: ~1.06 ms/exec on 8 cores (baseline 1.28-1.42 ms), rel err 3.9e-4.
"""

import numpy as np

import concourse.bass as bass
import concourse.mybir as mybir
import concourse.tile as tile
from concourse import bacc
from concourse.bass_utils import run_bass_kernel_spmd

# ---------------------------------------------------------------- constants
BATCH, SEQ, HID = 2, 4096, 2048
H, D, B = 32, 64, 256
NB = SEQ // B                    # 16 blocks
LAYER_IDX, N_LAYERS = 3, 12
EPS = 1e-5
N_CORES = 8
HG = 4                           # head groups (tensor parallel)
HL = H // HG                     # 8 local heads
NKT = HID // 128                 # 16 contraction tiles
NCH = 4                          # A2A chunks (4 blocks each)
TQ = 1024                        # phase-4 tokens per core (512 per batch)

F32 = mybir.dt.float32
FP16 = mybir.dt.float16
BF16 = mybir.dt.bfloat16
ACT = mybir.ActivationFunctionType
ALU = mybir.AluOpType

_cached_nc = None


def _decays_np():
    hr = np.arange(1, H + 1, dtype=np.float64)
    s = (1.0 / 2.0 ** (8.0 / H)) ** hr
    s = s * (1.0 - LAYER_IDX / (N_LAYERS - 1) + 1e-5)
    r = np.arange(1, B + 1, dtype=np.float64)
    q_dec = np.exp(-s[:, None] * r[None, :])                 # [H,B]
    k_dec = np.exp(-s[:, None] * (B - r)[None, :])           # [H,B]
    diff = r[:, None] - r[None, :]
    diag = np.where(diff[None] >= 0,
                    np.exp(-s[:, None, None] * diff[None]), 0.0)  # [H,B,B]
    blk = np.exp(-s * B)                                     # [H]
    return q_dec, k_dec, diag, blk


def _build(repeat=1, dbg=False):
    from contextlib import ExitStack

    nc = bacc.Bacc("TRN2", target_bir_lowering=False, debug=False,
                   num_devices=N_CORES)
    if dbg:
        dqk = nc.dram_tensor("dqk", [2, 128, 4, 512], FP16,
                             kind="ExternalOutput").ap()
        dvs = nc.dram_tensor("dvs", [128, 4, 512], FP16,
                             kind="ExternalOutput").ap()
        dks = nc.dram_tensor("dks", [128, 4, 2, 2, 128], FP16,
                             kind="ExternalOutput").ap()
        dst = nc.dram_tensor("dst", [128, 4, 2, 2, 2, B], FP16,
                             kind="ExternalOutput").ap()
        dal = nc.dram_tensor("dal", [N_CORES, HL * D, 128], FP16,
                             kind="ExternalOutput").ap()
        daq = nc.dram_tensor("daq", [N_CORES, HL * D, 128], FP16,
                             kind="ExternalOutput").ap()
        dat = nc.dram_tensor("dat", [128, NKT, 2 * 128], FP16,
                             kind="ExternalOutput").ap()

    hidT = nc.dram_tensor("hidT", [NB // 2, 128, NKT, 512], FP16,
                          kind="ExternalInput").ap()
    hidTq = nc.dram_tensor("hidTq", [128, NKT, TQ], FP16,
                           kind="ExternalInput").ap()
    w_qk = nc.dram_tensor("w_qk", [128, NKT, HL * 2 * D], FP16,
                          kind="ExternalInput").ap()
    w_v = nc.dram_tensor("w_v", [128, NKT, HL * D], FP16,
                         kind="ExternalInput").ap()
    w_gate = nc.dram_tensor("w_gate", [NKT, 128, NKT, 128], FP16,
                            kind="ExternalInput").ap()
    w_out = nc.dram_tensor("w_out", [128, NKT, HID], FP16,
                           kind="ExternalInput").ap()
    normw = nc.dram_tensor("normw", [HID], F32, kind="ExternalInput").ap()
    ddt = nc.dram_tensor("ddt", [HL, 2, 128, B], FP16,
                         kind="ExternalInput").ap()
    qdbc = nc.dram_tensor("qdbc", [128, HL // 2, 512], FP16,
                          kind="ExternalInput").ap()
    kdc = nc.dram_tensor("kdc", [128, HL // 2, 512], FP16,
                         kind="ExternalInput").ap()
    bdi = nc.dram_tensor("bdi", [HL, D, D], FP16, kind="ExternalInput").ap()
    y = nc.dram_tensor("y", [TQ, HID], F32, kind="ExternalOutput").ap()

    with tile.TileContext(nc) as tc, ExitStack() as top:
        constp = top.enter_context(tc.tile_pool(name="const", bufs=1))
        wp = top.enter_context(tc.tile_pool(name="wp", bufs=1))
        atp = top.enter_context(tc.tile_pool(name="atp", bufs=1))
        htp = top.enter_context(tc.tile_pool(name="htp", bufs=2))
        dramp = top.enter_context(tc.tile_pool(name="dram", bufs=1,
                                               space="DRAM"))

        normw_sb = constp.tile([128, NKT], F32)
        nc.sync.dma_start(out=normw_sb[:],
                          in_=normw.rearrange("(k p) -> p k", p=128))
        ddt_sb = constp.tile([128, HL, 2, B], FP16)
        nc.sync.dma_start(out=ddt_sb[:],
                          in_=ddt.rearrange("h jc p i -> p h jc i"))
        qd_sb = constp.tile([128, HL // 2, 512], FP16)
        nc.sync.dma_start(out=qd_sb[:], in_=qdbc[:])
        kd_sb = constp.tile([128, HL // 2, 512], FP16)
        nc.sync.dma_start(out=kd_sb[:], in_=kdc[:])
        bdi_sb = constp.tile([128, HL // 2, D], FP16)
        nc.sync.dma_start(
            out=bdi_sb[:],
            in_=bdi.rearrange("(hp two) d e -> (two d) hp e", two=2))
        ones_bf = constp.tile([128, 1], BF16)
        nc.vector.memset(ones_bf[:], 1.0)
        ones_f32 = constp.tile([1, 1], F32)
        nc.vector.memset(ones_f32[:], 1.0)
        eps_sb = constp.tile([1, 1], F32)
        nc.vector.memset(eps_sb[:], EPS)
        kv_sb = constp.tile([128, HL // 2, D], FP16)

        attn_loc = [dramp.tile([N_CORES, HL * D, 128], FP16, tag=f"al{m}",
                               name=f"attn_loc{m}")
                    for m in range(NCH)]
        attn_q = [dramp.tile([N_CORES, HL * D, 128], FP16, tag=f"aq{m}",
                             name=f"attn_q{m}")
                  for m in range(NCH)]
        gt_dram = dramp.tile([NKT, 128, TQ], FP16)
        attnTs = [atp.tile([128, NKT, 2 * 128], FP16, tag=f"attnT{m}",
                           name=f"attnT{m}")
                  for m in range(NCH)]

        for _rep in range(repeat):
            # ------------------------------------------------ phase 0: gate
            # streams w_gate gf-chunks on the sync queue; w_qk/w_v prefetch
            # runs concurrently on the scalar queue.
            w_qk_sb = wp.tile([128, NKT, HL * 2 * D], FP16, tag="wqk")
            w_v_sb = wp.tile([128, NKT, HL * D], FP16, tag="wv")
            nc.scalar.dma_start(out=w_v_sb[:], in_=w_v[:])
            for kq in range(4):
                nc.scalar.dma_start(out=w_qk_sb[:, kq * 4:(kq + 1) * 4, :],
                                    in_=w_qk[:, kq * 4:(kq + 1) * 4, :])

            hidT_tiles = {}

            def load_hidT(pr):
                t = htp.tile([128, NKT, 512], FP16, tag="hidT",
                             name=f"hidT_sb{pr % 2}")
                nc.scalar.dma_start(out=t[:], in_=hidT[pr])
                hidT_tiles[pr] = t

            load_hidT(0)
            load_hidT(1)

            with ExitStack() as ph0:
                hqp = ph0.enter_context(tc.tile_pool(name="hqp", bufs=1))
                wgp = ph0.enter_context(tc.tile_pool(name="wgp", bufs=3))
                gtsp = ph0.enter_context(tc.tile_pool(name="gtsp", bufs=2))
                gps = ph0.enter_context(
                    tc.tile_pool(name="gps", bufs=2, space="PSUM"))

                hidT_q = hqp.tile([128, NKT, TQ], FP16)
                wgs = {}

                def load_wg(gf):
                    wgs[gf] = wgp.tile([128, NKT, 128], FP16, tag="wg",
                                       name=f"wg{gf % 3}")
                    nc.sync.dma_start(out=wgs[gf][:], in_=w_gate[gf])

                load_wg(0)
                load_wg(1)
                for k in range(NKT):
                    nc.sync.dma_start(out=hidT_q[:, k, :], in_=hidTq[:, k, :])
                for gf in range(NKT):
                    if gf + 2 < NKT:
                        load_wg(gf + 2)
                    wg = wgs.pop(gf)
                    gt = gtsp.tile([128, TQ], FP16, tag="gt")
                    for c2 in range(2):
                        ps_g = gps.tile([128, 512], F32, tag="psg")
                        for k in range(NKT):
                            nc.tensor.matmul(
                                ps_g[:], wg[:, k, :],
                                hidT_q[:, k, c2 * 512:(c2 + 1) * 512],
                                start=(k == 0), stop=(k == NKT - 1))
                        nc.scalar.activation(gt[:, c2 * 512:(c2 + 1) * 512],
                                             ps_g[:], ACT.Sigmoid)
                    nc.sync.dma_start(out=gt_dram[gf], in_=gt[:])

            # ------------------------------------------- phase 1: attention
            nc.vector.memset(kv_sb[:], 0.0)
            with ExitStack() as ph1:
                vsp = ph1.enter_context(tc.tile_pool(name="vsp", bufs=2))
                qkp = ph1.enter_context(tc.tile_pool(name="qkp", bufs=1))
                ktdp = ph1.enter_context(tc.tile_pool(name="ktdp", bufs=2))
                stp = ph1.enter_context(tc.tile_pool(name="stp", bufs=2))
                scp = ph1.enter_context(tc.tile_pool(name="scp", bufs=3))
                ostg = ph1.enter_context(tc.tile_pool(name="ostg", bufs=3))
                qkps = ph1.enter_context(
                    tc.tile_pool(name="qkps", bufs=2, space="PSUM"))
                sps = ph1.enter_context(
                    tc.tile_pool(name="sps", bufs=2, space="PSUM"))
                ops = ph1.enter_context(
                    tc.tile_pool(name="ops", bufs=2, space="PSUM"))
                kvps = ph1.enter_context(
                    tc.tile_pool(name="kvps", bufs=2, space="PSUM"))

                for pr in range(NB // 2):        # block pairs, 512 tokens
                    hidT_sb = hidT_tiles.pop(pr)

                    # v_sd = silu(hidT.T @ w_v): [128 tok, 4 t4, 512 hd]
                    v_sd = vsp.tile([128, 4, HL * D], FP16, tag="vsd")
                    for t4 in range(4):
                        ps_v = qkps.tile([128, HL * D], F32, tag="psq")
                        for k in range(NKT):
                            nc.tensor.matmul(
                                ps_v[:],
                                hidT_sb[:, k, t4 * 128:(t4 + 1) * 128],
                                w_v_sb[:, k, :],
                                start=(k == 0), stop=(k == NKT - 1))
                        nc.scalar.activation(v_sd[:, t4, :], ps_v[:],
                                             ACT.Silu)

                    # S1: q/k projections for all 4 head pairs; k_dec-scaled
                    # copy kTd feeds DMA-engine transposes into ksd_all (no
                    # PE transposes, no psum bank, no ACT scales).
                    qTt = qkp.tile([128, 4, 512], FP16, tag="qTt")
                    kTt = qkp.tile([128, 4, 512], FP16, tag="kTt")
                    qdTt = qkp.tile([128, 4, 512], FP16, tag="qdTt")
                    kTd = ktdp.tile([128, 4, 512], FP16, tag="kTd")
                    ksd_all = stp.tile([128, 4, 2, 2, 128], FP16, tag="ksd")
                    sT_all = stp.tile([128, 4, 2, 2, 2, B], FP16, tag="sT")
                    # k first: the ksd DMA-transposes are the longest pole
                    for hp in range(HL // 2):
                        ps_k = qkps.tile([128, 512], F32, tag="psq")
                        for k in range(NKT):
                            nc.tensor.matmul(
                                ps_k[:],
                                w_qk_sb[:, k,
                                        512 + hp * 128:512 + (hp + 1) * 128],
                                hidT_sb[:, k, :],
                                start=(k == 0), stop=(k == NKT - 1))
                        nc.scalar.activation(kTt[:, hp, :], ps_k[:],
                                             ACT.Silu)
                        nc.vector.tensor_mul(kTd[:, hp, :], kTt[:, hp, :],
                                             kd_sb[:, hp, :])
                        for ib in range(2):
                            for jc in range(2):
                                nc.sync.dma_start_transpose(
                                    out=ksd_all[:, hp, ib, jc, :],
                                    in_=kTd[:, hp,
                                            ib * B + jc * 128:
                                            ib * B + (jc + 1) * 128])
                    for hp in range(HL // 2):
                        ps_q = qkps.tile([128, 512], F32, tag="psq")
                        for k in range(NKT):
                            nc.tensor.matmul(
                                ps_q[:],
                                w_qk_sb[:, k, hp * 128:(hp + 1) * 128],
                                hidT_sb[:, k, :],
                                start=(k == 0), stop=(k == NKT - 1))
                        nc.scalar.activation(qTt[:, hp, :], ps_q[:],
                                             ACT.Silu)
                        nc.vector.tensor_mul(qdTt[:, hp, :], qTt[:, hp, :],
                                             qd_sb[:, hp, :])

                    if pr + 2 < NB // 2:
                        load_hidT(pr + 2)

                    # S2: all scores + decay mult as one dense stream
                    for hp in range(HL // 2):
                        for hh in range(2):
                            h = hp * 2 + hh
                            pb = hh * D
                            for ib in range(2):
                                qT = qTt[pb:pb + D, hp,
                                         ib * B:(ib + 1) * B]
                                ps_s = sps.tile([128, 2, B], F32, tag="pss")
                                for jc in range(2):
                                    nc.tensor.matmul(
                                        ps_s[:, jc, :],
                                        kTt[pb:pb + D, hp,
                                            ib * B + jc * 128:
                                            ib * B + (jc + 1) * 128],
                                        qT, start=True, stop=True)
                                nc.vector.tensor_mul(
                                    sT_all[:, hp, hh, ib, :, :], ps_s[:],
                                    ddt_sb[:, h, :, :])

                    if dbg and pr == 0 and _rep == 0:
                        nc.sync.dma_start(out=dqk[0], in_=qTt[:])
                        nc.sync.dma_start(out=dqk[1], in_=kTt[:])
                        nc.sync.dma_start(out=dvs[:], in_=v_sd[:])
                        nc.sync.dma_start(out=dks[:], in_=ksd_all[:])
                        nc.sync.dma_start(out=dst[:], in_=sT_all[:])

                    # S3: o accumulation + kv recurrence, ib-major
                    for ib in range(2):
                        n = pr * 2 + ib
                        sl = 2 * (n % 4)
                        for hp in range(HL // 2):
                          for hh in range(2):
                            h = hp * 2 + hh
                            pb = hh * D
                            ps_o = ops.tile([D, B], F32, tag="pso")
                            for jc in range(2):
                                nc.tensor.matmul(
                                    ps_o[:],
                                    v_sd[:, ib * 2 + jc, h * D:(h + 1) * D],
                                    sT_all[:, hp, hh, ib, jc, :],
                                    start=(jc == 0), stop=False)
                            nc.tensor.matmul(
                                ps_o[:], kv_sb[pb:pb + D, hp, :],
                                qdTt[pb:pb + D, hp, ib * B:(ib + 1) * B],
                                start=False, stop=True)
                            o_sb = ostg.tile([D, B], FP16, tag="osb")
                            nc.vector.tensor_copy(o_sb[:], ps_o[:])
                            for half in range(2):
                                nc.sync.dma_start(
                                    out=attn_loc[n // 4][sl + half,
                                                         h * D:(h + 1) * D,
                                                         :],
                                    in_=o_sb[:, half * 128:
                                             (half + 1) * 128])

                            # kv <- bd*kv + (k*kd)^T @ v  (odd heads at
                            # partitions 64-127 via tile_position)
                            ps_kv = kvps.tile([128, D], F32, tag="pskv")
                            nc.tensor.matmul(
                                ps_kv[pb:pb + D, :],
                                bdi_sb[pb:pb + D, hp, :],
                                kv_sb[pb:pb + D, hp, :],
                                start=True, stop=False,
                                tile_position=(pb, pb))
                            for jc in range(2):
                                nc.tensor.matmul(
                                    ps_kv[pb:pb + D, :],
                                    ksd_all[:, hp, ib, jc,
                                            hh * D:(hh + 1) * D],
                                    v_sd[:, ib * 2 + jc, h * D:(h + 1) * D],
                                    start=False, stop=(jc == 1),
                                    tile_position=(0, pb))
                            nc.scalar.activation(kv_sb[pb:pb + D, hp, :],
                                                 ps_kv[pb:pb + D, :],
                                                 ACT.Copy)

                    if pr % 2 == 1:
                        m = pr // 2
                        nc.gpsimd.collective_compute(
                            "AllToAll", ALU.bypass,
                            replica_groups=[list(range(N_CORES))],
                            ins=[attn_loc[m][:].opt()],
                            outs=[attn_q[m][:].opt()])
                        for b2 in range(2):
                            nc.gpsimd.dma_start(
                                out=attnTs[m][:, :,
                                              b2 * 128:(b2 + 1) * 128],
                                in_=attn_q[m][4 * b2:4 * b2 + 4].rearrange(
                                    "gg (kk p) t -> p (gg kk) t", kk=4))
                        if dbg and m == 0 and _rep == 0:
                            nc.sync.dma_start(out=dal[:], in_=attn_loc[0][:])
                            nc.sync.dma_start(out=daq[:], in_=attn_q[0][:])
                            nc.gpsimd.dma_start(out=dat[:], in_=attnTs[0][:])

            # ------------------------------------------------ tail: phase 4
            with ExitStack() as ph4:
                glp = ph4.enter_context(tc.tile_pool(name="glp", bufs=2))
                sqp = ph4.enter_context(tc.tile_pool(name="sqp", bufs=2))
                rsp = ph4.enter_context(tc.tile_pool(name="rsp", bufs=2))
                wop = ph4.enter_context(tc.tile_pool(name="wop", bufs=2))
                ystg = ph4.enter_context(tc.tile_pool(name="ystg", bufs=2))
                ssps = ph4.enter_context(
                    tc.tile_pool(name="ssps", bufs=1, space="PSUM"))
                yps = ph4.enter_context(
                    tc.tile_pool(name="yps", bufs=2, space="PSUM"))

                rstd_t = [None] * NCH

                def rstd_pre(ms):
                    """sumsq (interleaved across chunks) -> rstd_t."""
                    ps_ss = {m: ssps.tile([1, 2 * 128], F32,
                                          tag=f"psss{m % 3}",
                                          name=f"ps_ss{m % 3}")
                             for m in ms}
                    for k in range(NKT):
                        for m in ms:
                            sq = sqp.tile([128, 2 * 128], BF16,
                                          tag=f"sq{m % 3}",
                                          name=f"sq{m % 3}")
                            if (k + m) % 2 == 0:
                                nc.vector.tensor_mul(
                                    sq[:], attnTs[m][:, k, :],
                                    attnTs[m][:, k, :])
                            else:
                                nc.scalar.activation(
                                    sq[:], attnTs[m][:, k, :], ACT.Square)
                            nc.tensor.matmul(ps_ss[m][:], ones_bf[:], sq[:],
                                             start=(k == 0),
                                             stop=(k == NKT - 1))
                    for m in ms:
                        sdev = rsp.tile([1, 2 * 128], F32, tag="sdev")
                        nc.scalar.activation(sdev[:], ps_ss[m][:], ACT.Sqrt,
                                             bias=eps_sb[0:1, 0:1],
                                             scale=1.0 / HID)
                        rstd = rsp.tile([1, 2 * 128], F32, tag="rstd")
                        nc.vector.reciprocal(rstd[:], sdev[:])
                        ps_rt = ssps.tile([128, 2], F32, tag="psrt")
                        for tg in range(2):
                            nc.tensor.matmul(
                                ps_rt[:, tg:tg + 1],
                                rstd[0:1, tg * 128:(tg + 1) * 128],
                                ones_f32[0:1, 0:1], start=True, stop=True)
                        rt = rsp.tile([128, 2], F32, tag=f"rt{m}")
                        nc.vector.tensor_copy(rt[:], ps_rt[:])
                        rstd_t[m] = rt

                def gate_mult(m):
                    """attnT *= normw * gate."""
                    attnT = attnTs[m]
                    gtl = glp.tile([128, NKT, 2 * 128], FP16, tag="gtl")
                    for b2 in range(2):
                        nc.sync.dma_start(
                            out=gtl[:, :, b2 * 128:(b2 + 1) * 128],
                            in_=gt_dram[:, :,
                                        b2 * 512 + m * 128:
                                        b2 * 512 + (m + 1) * 128].rearrange(
                                            "kk p t -> p kk t"))
                    for k in range(NKT):
                        nc.vector.scalar_tensor_tensor(
                            out=attnT[:, k, :], in0=attnT[:, k, :],
                            scalar=normw_sb[:, k:k + 1], in1=gtl[:, k, :],
                            op0=ALU.mult, op1=ALU.mult)

                def out_proj(nn, m, wo):
                    attnT = attnTs[m]
                    for tg in range(2):
                        ps_y = yps.tile([128, 512], F32, tag="psy")
                        for k in range(NKT):
                            nc.tensor.matmul(
                                ps_y[:],
                                attnT[:, k, tg * 128:(tg + 1) * 128],
                                wo[:, k, :],
                                start=(k == 0), stop=(k == NKT - 1))
                        y_sb = ystg.tile([128, 512], F32, tag="ysb")
                        nc.scalar.mul(y_sb[:], ps_y[:],
                                      rstd_t[m][:, tg:tg + 1])
                        nc.sync.dma_start(
                            out=y[tg * 512 + m * 128:
                                  tg * 512 + (m + 1) * 128,
                                  nn * 512:(nn + 1) * 512],
                            in_=y_sb[:])

                # chunks 0-2 first (~80us of PE work hiding the chunk-3
                # A2A), then chunk 3 with w_out re-streamed.
                rstd_pre(range(NCH - 1))
                for m in range(NCH - 1):
                    gate_mult(m)
                for nn in range(4):
                    wo = wop.tile([128, NKT, 512], FP16, tag="wo")
                    nc.scalar.dma_start(
                        out=wo[:], in_=w_out[:, :, nn * 512:(nn + 1) * 512])
                    for m in range(NCH - 1):
                        out_proj(nn, m, wo)
                    if nn == 0:
                        rstd_pre([NCH - 1])
                        gate_mult(NCH - 1)
                for nn in range(4):
                    wo = wop.tile([128, NKT, 512], FP16, tag="wo")
                    nc.scalar.dma_start(
                        out=wo[:], in_=w_out[:, :, nn * 512:(nn + 1) * 512])
                    out_proj(nn, NCH - 1, wo)

    nc.compile()
    return nc


def _qd_pair(qd_l):
    """[128, HL//2, 512] fp16: rows hh*64+d hold q_dec[2hp+hh] tiled over
    both 256-token blocks."""
    out = np.zeros((128, HL // 2, 512), np.float16)
    for hp in range(HL // 2):
        for hh in range(2):
            row = np.tile(qd_l[2 * hp + hh], 2)
            out[hh * 64:(hh + 1) * 64, hp, :] = row[None, :]
    return out


def _in_maps(hidden_states, w_qkv, norm_weight, w_gate, w_out):
    q_dec, k_dec, diag, blk = _decays_np()
    f16 = lambda a: np.ascontiguousarray(a, dtype=np.float16)
    f32 = lambda a: np.ascontiguousarray(a, dtype=np.float32)

    w_qkv_r = np.asarray(w_qkv).reshape(HID, H, 3, D)
    # hidT_p[b]: [8 pr, 128 p, 16 k, 512 t]
    hidT_all = [
        f16(np.asarray(hidden_states[b]).reshape(8, 512, NKT, 128)
            .transpose(0, 3, 2, 1))
        for b in range(BATCH)
    ]
    # hidTq per core: [128 p, 16 k, 1024 lt], lt = b2*512 + mm*128 + t
    hs_r = np.asarray(hidden_states).reshape(BATCH, 32, 128, NKT, 128)
    w_gate_p = f16(np.asarray(w_gate).reshape(NKT, 128, NKT, 128)
                   .transpose(2, 1, 0, 3))
    w_out_p = f16(np.asarray(w_out).reshape(NKT, 128, HID).transpose(1, 0, 2))
    maps = []
    for c in range(N_CORES):
        b, g = divmod(c, HG)
        hs = slice(g * HL, (g + 1) * HL)
        # my half-blocks: 8m + c of each batch; hs_r[b2][hb] is
        # [4 mm, 128 t, 16 k, 128 p] -> [128 p, 16 k, (b2 mm t)]
        hb = [8 * mm + c for mm in range(NCH)]
        hq = np.stack([hs_r[b2][hb] for b2 in range(BATCH)])  # [2,4,128,16,128]
        hq = hq.transpose(4, 3, 0, 1, 2).reshape(128, NKT, TQ)
        wq = np.concatenate(
            [np.ascontiguousarray(w_qkv_r[:, hs, 0, :]).reshape(HID, HL * D),
             np.ascontiguousarray(w_qkv_r[:, hs, 1, :]).reshape(HID, HL * D)],
            axis=1).reshape(NKT, 128, HL * 2 * D).transpose(1, 0, 2)
        wv = (np.ascontiguousarray(w_qkv_r[:, hs, 2, :])
              .reshape(NKT, 128, HL * D).transpose(1, 0, 2))
        maps.append({
            "hidT": hidT_all[b],
            "hidTq": f16(hq),
            "w_qk": f16(wq),
            "w_v": f16(wv),
            "w_gate": w_gate_p,
            "w_out": w_out_p,
            "normw": f32(norm_weight),
            "ddt": f16(diag[hs].transpose(0, 2, 1).reshape(HL, 2, 128, B)),
            "qdbc": _qd_pair(q_dec[hs]),
            "kdc": _qd_pair(k_dec[hs]),
            "bdi": f16(np.eye(D)[None] * blk[hs][:, None, None]),
        })
    return maps


def _gather(res):
    """res[c]["y"] rows are lt = b2*512 + mm*128 + t; global token
    (b2, (8*mm + c)*128 + t)."""
    out = np.empty((BATCH, SEQ, HID), dtype=np.float32)
    for c in range(N_CORES):
        yv = np.asarray(res[c]["y"]).reshape(BATCH, NCH, 128, HID)
        for mm in range(NCH):
            hb = 8 * mm + c
            out[:, hb * 128:(hb + 1) * 128, :] = yv[:, mm]
    return out


def kernel(hidden_states, w_qkv, norm_weight, w_gate, w_out):
    global _cached_nc
    if _cached_nc is None:
        _cached_nc = _build()
    nc = _cached_nc

    maps = _in_maps(hidden_states, w_qkv, norm_weight, w_gate, w_out)
    res = run_bass_kernel_spmd(nc, maps, list(range(N_CORES)))
    return _gather(res.results)


# revision 81
# speedup vs baseline: 1.2898x; 1.2898x over previous
"""MiniMax-Text-01 lightning attention layer on 8 Trainium2 NeuronCores (v2).

Sharding: core c = 4*b + g runs attention for batch b, heads [8g, 8g+8).
Phase 4 (RMSNorm/gate/out-proj) is sharded over 1024 INTERLEAVED tokens from
BOTH batches (half-blocks {8m + c : m=0..3} of 128 tokens in each batch), so
the 8-rank AllToAll carries no duplicated data: chunk m (blocks 4m..4m+3)
becomes ready at 25/50/75/100% of the attention scan and is exchanged
immediately, hiding the collective under compute.

Host pre-packs every input to fp16 in the exact SBUF layout (including the
pre-transposed hidden states), so the device runs almost pure matmul streams:
  phase 0: gt = sigmoid(w_gate.T @ hidT_q) -> DRAM fp16, while w_qk/w_v
           prefetch streams underneath on another DMA queue.
  phase 1: 8 block-pairs; per pair: v_sd / qT / kT projections (fp16, FWL)
           and 16 head-blocks of block-recurrent attention.  The per-head KV
           state for odd heads lives at partitions 64-127 via tile_position
           col-offset matmuls (no SBUF shift DMAs).  After pair 2m+1: A2A
           chunk m + assembly DMA into SBUF (gpsimd queue).
  tail:    per chunk: sumsq (ones-matmul) -> rstd; attnT *= normw*gate;
           y = (attnT).T @ w_out * rstd -> DRAM f32 (w_out streamed nn-major).

Measured: ~1.06 ms/exec on 8 cores (baseline 1.28-1.42 ms), rel err 3.9e-4.
"""

import numpy as np

import concourse.bass as bass
import concourse.mybir as mybir
import concourse.tile as tile
from concourse import bacc
from concourse.bass_utils import run_bass_kernel_spmd

# ---------------------------------------------------------------- constants
BATCH, SEQ, HID = 2, 4096, 2048
H, D, B = 32, 64, 256
NB = SEQ // B                    # 16 blocks
LAYER_IDX, N_LAYERS = 3, 12
EPS = 1e-5
N_CORES = 8
HG = 4                           # head groups (tensor parallel)
HL = H // HG                     # 8 local heads
NKT = HID // 128                 # 16 contraction tiles
NCH = 4                          # A2A chunks (4 blocks each)
TQ = 1024                        # phase-4 tokens per core (512 per batch)

F32 = mybir.dt.float32
FP16 = mybir.dt.float16
BF16 = mybir.dt.bfloat16
ACT = mybir.ActivationFunctionType
ALU = mybir.AluOpType

_cached_nc = None


def _decays_np():
    hr = np.arange(1, H + 1, dtype=np.float64)
    s = (1.0 / 2.0 ** (8.0 / H)) ** hr
    s = s * (1.0 - LAYER_IDX / (N_LAYERS - 1) + 1e-5)
    r = np.arange(1, B + 1, dtype=np.float64)
    q_dec = np.exp(-s[:, None] * r[None, :])                 # [H,B]
    k_dec = np.exp(-s[:, None] * (B - r)[None, :])           # [H,B]
    diff = r[:, None] - r[None, :]
    diag = np.where(diff[None] >= 0,
                    np.exp(-s[:, None, None] * diff[None]), 0.0)  # [H,B,B]
    blk = np.exp(-s * B)                                     # [H]
    return q_dec, k_dec, diag, blk


def _build(repeat=1, dbg=False):
    from contextlib import ExitStack

    nc = bacc.Bacc("TRN2", target_bir_lowering=False, debug=False,
                   num_devices=N_CORES)
    if dbg:
        dqk = nc.dram_tensor("dqk", [2, 128, 4, 512], FP16,
                             kind="ExternalOutput").ap()
        dvs = nc.dram_tensor("dvs", [128, 4, 512], FP16,
                             kind="ExternalOutput").ap()
        dks = nc.dram_tensor("dks", [128, 4, 2, 2, 128], FP16,
                             kind="ExternalOutput").ap()
        dst = nc.dram_tensor("dst", [128, 4, 2, 2, 2, B], FP16,
                             kind="ExternalOutput").ap()
        dal = nc.dram_tensor("dal", [N_CORES, HL * D, 128], FP16,
                             kind="ExternalOutput").ap()
        daq = nc.dram_tensor("daq", [N_CORES, HL * D, 128], FP16,
                             kind="ExternalOutput").ap()
        dat = nc.dram_tensor("dat", [128, NKT, 2 * 128], FP16,
                             kind="ExternalOutput").ap()

    hidT = nc.dram_tensor("hidT", [NB // 2, 128, NKT, 512], FP16,
                          kind="ExternalInput").ap()
    hidTq = nc.dram_tensor("hidTq", [128, NKT, TQ], FP16,
                           kind="ExternalInput").ap()
    w_qk = nc.dram_tensor("w_qk", [128, NKT, HL * 2 * D], FP16,
                          kind="ExternalInput").ap()
    w_v = nc.dram_tensor("w_v", [128, NKT, HL * D], FP16,
                         kind="ExternalInput").ap()
    w_gate = nc.dram_tensor("w_gate", [NKT, 128, NKT, 128], FP16,
                            kind="ExternalInput").ap()
    w_out = nc.dram_tensor("w_out", [128, NKT, HID], FP16,
                           kind="ExternalInput").ap()
    normw = nc.dram_tensor("normw", [HID], F32, kind="ExternalInput").ap()
    ddt = nc.dram_tensor("ddt", [HL, 2, 128, B], FP16,
                         kind="ExternalInput").ap()
    qdbc = nc.dram_tensor("qdbc", [128, HL // 2, 512], FP16,
                          kind="ExternalInput").ap()
    kdc = nc.dram_tensor("kdc", [128, HL // 2, 512], FP16,
                         kind="ExternalInput").ap()
    bdi = nc.dram_tensor("bdi", [HL, D, D], FP16, kind="ExternalInput").ap()
    y = nc.dram_tensor("y", [TQ, HID], F32, kind="ExternalOutput").ap()

    with tile.TileContext(nc) as tc, ExitStack() as top:
        constp = top.enter_context(tc.tile_pool(name="const", bufs=1))
        wp = top.enter_context(tc.tile_pool(name="wp", bufs=1))
        atp = top.enter_context(tc.tile_pool(name="atp", bufs=1))
        htp = top.enter_context(tc.tile_pool(name="htp", bufs=2))
        dramp = top.enter_context(tc.tile_pool(name="dram", bufs=1,
                                               space="DRAM"))

        normw_sb = constp.tile([128, NKT], F32)
        nc.sync.dma_start(out=normw_sb[:],
                          in_=normw.rearrange("(k p) -> p k", p=128))
        ddt_sb = constp.tile([128, HL, 2, B], FP16)
        nc.sync.dma_start(out=ddt_sb[:],
                          in_=ddt.rearrange("h jc p i -> p h jc i"))
        qd_sb = constp.tile([128, HL // 2, 512], FP16)
        nc.sync.dma_start(out=qd_sb[:], in_=qdbc[:])
        kd_sb = constp.tile([128, HL // 2, 512], FP16)
        nc.sync.dma_start(out=kd_sb[:], in_=kdc[:])
        bdi_sb = constp.tile([128, HL // 2, D], FP16)
        nc.sync.dma_start(
            out=bdi_sb[:],
            in_=bdi.rearrange("(hp two) d e -> (two d) hp e", two=2))
        ones_bf = constp.tile([128, 1], BF16)
        nc.vector.memset(ones_bf[:], 1.0)
        ones_f32 = constp.tile([1, 1], F32)
        nc.vector.memset(ones_f32[:], 1.0)
        eps_sb = constp.tile([1, 1], F32)
        nc.vector.memset(eps_sb[:], EPS)
        kv_sb = constp.tile([128, HL // 2, D], FP16)

        attn_loc = [dramp.tile([N_CORES, HL * D, 128], FP16, tag=f"al{m}",
                               name=f"attn_loc{m}")
                    for m in range(NCH)]
        attn_q = [dramp.tile([N_CORES, HL * D, 128], FP16, tag=f"aq{m}",
                             name=f"attn_q{m}")
                  for m in range(NCH)]
        gt_dram = dramp.tile([NKT, 128, TQ], FP16)
        attnTs = [atp.tile([128, NKT, 2 * 128], FP16, tag=f"attnT{m}",
                           name=f"attnT{m}")
                  for m in range(NCH)]

        for _rep in range(repeat):
            # ------------------------------------------------ phase 0: gate
            # streams w_gate gf-chunks on the sync queue; w_qk/w_v prefetch
            # runs concurrently on the scalar queue.
            w_qk_sb = wp.tile([128, NKT, HL * 2 * D], FP16, tag="wqk")
            w_v_sb = wp.tile([128, NKT, HL * D], FP16, tag="wv")
            nc.scalar.dma_start(out=w_v_sb[:], in_=w_v[:])
            for kq in range(4):
                nc.scalar.dma_start(out=w_qk_sb[:, kq * 4:(kq + 1) * 4, :],
                                    in_=w_qk[:, kq * 4:(kq + 1) * 4, :])

            hidT_tiles = {}

            def load_hidT(pr):
                t = htp.tile([128, NKT, 512], FP16, tag="hidT",
                             name=f"hidT_sb{pr % 2}")
                nc.scalar.dma_start(out=t[:], in_=hidT[pr])
                hidT_tiles[pr] = t

            load_hidT(0)
            load_hidT(1)

            with ExitStack() as ph0:
                hqp = ph0.enter_context(tc.tile_pool(name="hqp", bufs=1))
                wgp = ph0.enter_context(tc.tile_pool(name="wgp", bufs=3))
                gtsp = ph0.enter_context(tc.tile_pool(name="gtsp", bufs=2))
                gps = ph0.enter_context(
                    tc.tile_pool(name="gps", bufs=2, space="PSUM"))

                hidT_q = hqp.tile([128, NKT, TQ], FP16)
                wgs = {}

                def load_wg(gf):
                    wgs[gf] = wgp.tile([128, NKT, 128], FP16, tag="wg",
                                       name=f"wg{gf % 3}")
                    nc.sync.dma_start(out=wgs[gf][:], in_=w_gate[gf])

                load_wg(0)
                load_wg(1)
                for k in range(NKT):
                    nc.sync.dma_start(out=hidT_q[:, k, :], in_=hidTq[:, k, :])
                for gf in range(NKT):
                    if gf + 2 < NKT:
                        load_wg(gf + 2)
                    wg = wgs.pop(gf)
                    gt = gtsp.tile([128, TQ], FP16, tag="gt")
                    for c2 in range(2):
                        ps_g = gps.tile([128, 512], F32, tag="psg")
                        for k in range(NKT):
                            nc.tensor.matmul(
                                ps_g[:], wg[:, k, :],
                                hidT_q[:, k, c2 * 512:(c2 + 1) * 512],
                                start=(k == 0), stop=(k == NKT - 1))
                        nc.scalar.activation(gt[:, c2 * 512:(c2 + 1) * 512],
                                             ps_g[:], ACT.Sigmoid)
                    nc.sync.dma_start(out=gt_dram[gf], in_=gt[:])

            # ------------------------------------------- phase 1: attention
            nc.vector.memset(kv_sb[:], 0.0)
            with ExitStack() as ph1:
                vsp = ph1.enter_context(tc.tile_pool(name="vsp", bufs=2))
                qkp = ph1.enter_context(tc.tile_pool(name="qkp", bufs=1))
                ktdp = ph1.enter_context(tc.tile_pool(name="ktdp", bufs=2))
                stp = ph1.enter_context(tc.tile_pool(name="stp", bufs=2))
                scp = ph1.enter_context(tc.tile_pool(name="scp", bufs=3))
                ostg = ph1.enter_context(tc.tile_pool(name="ostg", bufs=3))
                qkps = ph1.enter_context(
                    tc.tile_pool(name="qkps", bufs=2, space="PSUM"))
                sps = ph1.enter_context(
                    tc.tile_pool(name="sps", bufs=2, space="PSUM"))
                ops = ph1.enter_context(
                    tc.tile_pool(name="ops", bufs=2, space="PSUM"))
                kvps = ph1.enter_context(
                    tc.tile_pool(name="kvps", bufs=2, space="PSUM"))

                for pr in range(NB // 2):        # block pairs, 512 tokens
                    hidT_sb = hidT_tiles.pop(pr)

                    # v_sd = silu(hidT.T @ w_v): [128 tok, 4 t4, 512 hd]
                    v_sd = vsp.tile([128, 4, HL * D], FP16, tag="vsd")
                    for t4 in range(4):
                        ps_v = qkps.tile([128, HL * D], F32, tag="psq")
                        for k in range(NKT):
                            nc.tensor.matmul(
                                ps_v[:],
                                hidT_sb[:, k, t4 * 128:(t4 + 1) * 128],
                                w_v_sb[:, k, :],
                                start=(k == 0), stop=(k == NKT - 1))
                        nc.scalar.activation(v_sd[:, t4, :], ps_v[:],
                                             ACT.Silu)

                    # S1: q/k projections for all 4 head pairs; k_dec-scaled
                    # copy kTd feeds DMA-engine transposes into ksd_all (no
                    # PE transposes, no psum bank, no ACT scales).
                    qTt = qkp.tile([128, 4, 512], FP16, tag="qTt")
                    kTt = qkp.tile([128, 4, 512], FP16, tag="kTt")
                    qdTt = qkp.tile([128, 4, 512], FP16, tag="qdTt")
                    kTd = ktdp.tile([128, 4, 512], FP16, tag="kTd")
                    ksd_all = stp.tile([128, 4, 2, 2, 128], FP16, tag="ksd")
                    sT_all = stp.tile([128, 4, 2, 2, 2, B], FP16, tag="sT")
                    # k first: the ksd DMA-transposes are the longest pole
                    for hp in range(HL // 2):
                        ps_k = qkps.tile([128, 512], F32, tag="psq")
                        for k in range(NKT):
                            nc.tensor.matmul(
                                ps_k[:],
                                w_qk_sb[:, k,
                                        512 + hp * 128:512 + (hp + 1) * 128],
                                hidT_sb[:, k, :],
                                start=(k == 0), stop=(k == NKT - 1))
                        nc.scalar.activation(kTt[:, hp, :], ps_k[:],
                                             ACT.Silu)
                        nc.vector.tensor_mul(kTd[:, hp, :], kTt[:, hp, :],
                                             kd_sb[:, hp, :])
                        for ib in range(2):
                            for jc in range(2):
                                nc.sync.dma_start_transpose(
                                    out=ksd_all[:, hp, ib, jc, :],
                                    in_=kTd[:, hp,
                                            ib * B + jc * 128:
                                            ib * B + (jc + 1) * 128])
                    for hp in range(HL // 2):
                        ps_q = qkps.tile([128, 512], F32, tag="psq")
                        for k in range(NKT):
                            nc.tensor.matmul(
                                ps_q[:],
                                w_qk_sb[:, k, hp * 128:(hp + 1) * 128],
                                hidT_sb[:, k, :],
                                start=(k == 0), stop=(k == NKT - 1))
                        nc.scalar.activation(qTt[:, hp, :], ps_q[:],
                                             ACT.Silu)
                        nc.vector.tensor_mul(qdTt[:, hp, :], qTt[:, hp, :],
                                             qd_sb[:, hp, :])

                    if pr + 2 < NB // 2:
                        load_hidT(pr + 2)

                    # S2: all scores + decay mult as one dense stream
                    for hp in range(HL // 2):
                        for hh in range(2):
                            h = hp * 2 + hh
                            pb = hh * D
                            for ib in range(2):
                                qT = qTt[pb:pb + D, hp,
                                         ib * B:(ib + 1) * B]
                                ps_s = sps.tile([128, 2, B], F32, tag="pss")
                                for jc in range(2):
                                    nc.tensor.matmul(
                                        ps_s[:, jc, :],
                                        kTt[pb:pb + D, hp,
                                            ib * B + jc * 128:
                                            ib * B + (jc + 1) * 128],
                                        qT, start=True, stop=True)
                                nc.vector.tensor_mul(
                                    sT_all[:, hp, hh, ib, :, :], ps_s[:],
                                    ddt_sb[:, h, :, :])

                    if dbg and pr == 0 and _rep == 0:
                        nc.sync.dma_start(out=dqk[0], in_=qTt[:])
                        nc.sync.dma_start(out=dqk[1], in_=kTt[:])
                        nc.sync.dma_start(out=dvs[:], in_=v_sd[:])
                        nc.sync.dma_start(out=dks[:], in_=ksd_all[:])
                        nc.sync.dma_start(out=dst[:], in_=sT_all[:])

                    # S3: o accumulation + kv recurrence, ib-major
                    for ib in range(2):
                        n = pr * 2 + ib
                        sl = 2 * (n % 4)
                        for hp in range(HL // 2):
                          for hh in range(2):
                            h = hp * 2 + hh
                            pb = hh * D
                            ps_o = ops.tile([D, B], F32, tag="pso")
                            for jc in range(2):
                                nc.tensor.matmul(
                                    ps_o[:],
                                    v_sd[:, ib * 2 + jc, h * D:(h + 1) * D],
                                    sT_all[:, hp, hh, ib, jc, :],
                                    start=(jc == 0), stop=False)
                            nc.tensor.matmul(
                                ps_o[:], kv_sb[pb:pb + D, hp, :],
                                qdTt[pb:pb + D, hp, ib * B:(ib + 1) * B],
                                start=False, stop=True)
                            o_sb = ostg.tile([D, B], FP16, tag="osb")
                            if ib == 0:
                                nc.scalar.activation(o_sb[:], ps_o[:],
                                                     ACT.Copy)
                            else:
                                nc.vector.tensor_copy(o_sb[:], ps_o[:])
                            for half in range(2):
                                nc.sync.dma_start(
                                    out=attn_loc[n // 4][sl + half,
                                                         h * D:(h + 1) * D,
                                                         :],
                                    in_=o_sb[:, half * 128:
                                             (half + 1) * 128])

                            # kv <- bd*kv + (k*kd)^T @ v  (odd heads at
                            # partitions 64-127 via tile_position)
                            ps_kv = kvps.tile([128, D], F32, tag="pskv")
                            nc.tensor.matmul(
                                ps_kv[pb:pb + D, :],
                                bdi_sb[pb:pb + D, hp, :],
                                kv_sb[pb:pb + D, hp, :],
                                start=True, stop=False,
                                tile_position=(pb, pb))
                            for jc in range(2):
                                nc.tensor.matmul(
                                    ps_kv[pb:pb + D, :],
                                    ksd_all[:, hp, ib, jc,
                                            hh * D:(hh + 1) * D],
                                    v_sd[:, ib * 2 + jc, h * D:(h + 1) * D],
                                    start=False, stop=(jc == 1),
                                    tile_position=(0, pb))
                            nc.scalar.activation(kv_sb[pb:pb + D, hp, :],
                                                 ps_kv[pb:pb + D, :],
                                                 ACT.Copy)

                    if pr % 2 == 1:
                        m = pr // 2
                        nc.gpsimd.collective_compute(
                            "AllToAll", ALU.bypass,
                            replica_groups=[list(range(N_CORES))],
                            ins=[attn_loc[m][:].opt()],
                            outs=[attn_q[m][:].opt()])
                        for b2 in range(2):
                            nc.gpsimd.dma_start(
                                out=attnTs[m][:, :,
                                              b2 * 128:(b2 + 1) * 128],
                                in_=attn_q[m][4 * b2:4 * b2 + 4].rearrange(
                                    "gg (kk p) t -> p (gg kk) t", kk=4))
                        if dbg and m == 0 and _rep == 0:
                            nc.sync.dma_start(out=dal[:], in_=attn_loc[0][:])
                            nc.sync.dma_start(out=daq[:], in_=attn_q[0][:])
                            nc.gpsimd.dma_start(out=dat[:], in_=attnTs[0][:])

            # ------------------------------------------------ tail: phase 4
            with ExitStack() as ph4:
                glp = ph4.enter_context(tc.tile_pool(name="glp", bufs=2))
                sqp = ph4.enter_context(tc.tile_pool(name="sqp", bufs=2))
                rsp = ph4.enter_context(tc.tile_pool(name="rsp", bufs=2))
                wop = ph4.enter_context(tc.tile_pool(name="wop", bufs=2))
                ystg = ph4.enter_context(tc.tile_pool(name="ystg", bufs=2))
                ssps = ph4.enter_context(
                    tc.tile_pool(name="ssps", bufs=1, space="PSUM"))
                yps = ph4.enter_context(
                    tc.tile_pool(name="yps", bufs=2, space="PSUM"))

                rstd_t = [None] * NCH

                def rstd_pre(ms):
                    """sumsq (interleaved across chunks) -> rstd_t."""
                    ps_ss = {m: ssps.tile([1, 2 * 128], F32,
                                          tag=f"psss{m % 3}",
                                          name=f"ps_ss{m % 3}")
                             for m in ms}
                    for k in range(NKT):
                        for m in ms:
                            sq = sqp.tile([128, 2 * 128], BF16,
                                          tag=f"sq{m % 3}",
                                          name=f"sq{m % 3}")
                            if (k + m) % 2 == 0:
                                nc.vector.tensor_mul(
                                    sq[:], attnTs[m][:, k, :],
                                    attnTs[m][:, k, :])
                            else:
                                nc.scalar.activation(
                                    sq[:], attnTs[m][:, k, :], ACT.Square)
                            nc.tensor.matmul(ps_ss[m][:], ones_bf[:], sq[:],
                                             start=(k == 0),
                                             stop=(k == NKT - 1))
                    for m in ms:
                        sdev = rsp.tile([1, 2 * 128], F32, tag="sdev")
                        nc.scalar.activation(sdev[:], ps_ss[m][:], ACT.Sqrt,
                                             bias=eps_sb[0:1, 0:1],
                                             scale=1.0 / HID)
                        rstd = rsp.tile([1, 2 * 128], F32, tag="rstd")
                        nc.vector.reciprocal(rstd[:], sdev[:])
                        ps_rt = ssps.tile([128, 2], F32, tag="psrt")
                        for tg in range(2):
                            nc.tensor.matmul(
                                ps_rt[:, tg:tg + 1],
                                rstd[0:1, tg * 128:(tg + 1) * 128],
                                ones_f32[0:1, 0:1], start=True, stop=True)
                        rt = rsp.tile([128, 2], F32, tag=f"rt{m}")
                        nc.vector.tensor_copy(rt[:], ps_rt[:])
                        rstd_t[m] = rt

                def gate_mult(m):
                    """attnT *= normw * gate."""
                    attnT = attnTs[m]
                    gtl = glp.tile([128, NKT, 2 * 128], FP16, tag="gtl")
                    for b2 in range(2):
                        nc.sync.dma_start(
                            out=gtl[:, :, b2 * 128:(b2 + 1) * 128],
                            in_=gt_dram[:, :,
                                        b2 * 512 + m * 128:
                                        b2 * 512 + (m + 1) * 128].rearrange(
                                            "kk p t -> p kk t"))
                    for k in range(NKT):
                        nc.vector.scalar_tensor_tensor(
                            out=attnT[:, k, :], in0=attnT[:, k, :],
                            scalar=normw_sb[:, k:k + 1], in1=gtl[:, k, :],
                            op0=ALU.mult, op1=ALU.mult)

                def out_proj(nn, m, wo):
                    attnT = attnTs[m]
                    for tg in range(2):
                        ps_y = yps.tile([128, 512], F32, tag="psy")
                        for k in range(NKT):
                            nc.tensor.matmul(
                                ps_y[:],
                                attnT[:, k, tg * 128:(tg + 1) * 128],
                                wo[:, k, :],
                                start=(k == 0), stop=(k == NKT - 1))
                        y_sb = ystg.tile([128, 512], F32, tag="ysb")
                        nc.scalar.mul(y_sb[:], ps_y[:],
                                      rstd_t[m][:, tg:tg + 1])
                        nc.sync.dma_start(
                            out=y[tg * 512 + m * 128:
                                  tg * 512 + (m + 1) * 128,
                                  nn * 512:(nn + 1) * 512],
                            in_=y_sb[:])

                # chunks 0-2 first (~80us of PE work hiding the chunk-3
                # A2A), then chunk 3 with w_out re-streamed.
                rstd_pre(range(NCH - 1))
                for m in range(NCH - 1):
                    gate_mult(m)
                for nn in range(4):
                    wo = wop.tile([128, NKT, 512], FP16, tag="wo")
                    nc.scalar.dma_start(
                        out=wo[:], in_=w_out[:, :, nn * 512:(nn + 1) * 512])
                    for m in range(NCH - 1):
                        out_proj(nn, m, wo)
                    if nn == 0:
                        rstd_pre([NCH - 1])
                        gate_mult(NCH - 1)
                for nn in range(4):
                    wo = wop.tile([128, NKT, 512], FP16, tag="wo")
                    nc.scalar.dma_start(
                        out=wo[:], in_=w_out[:, :, nn * 512:(nn + 1) * 512])
                    out_proj(nn, NCH - 1, wo)

    nc.compile()
    return nc


def _qd_pair(qd_l):
    """[128, HL//2, 512] fp16: rows hh*64+d hold q_dec[2hp+hh] tiled over
    both 256-token blocks."""
    out = np.zeros((128, HL // 2, 512), np.float16)
    for hp in range(HL // 2):
        for hh in range(2):
            row = np.tile(qd_l[2 * hp + hh], 2)
            out[hh * 64:(hh + 1) * 64, hp, :] = row[None, :]
    return out


def _in_maps(hidden_states, w_qkv, norm_weight, w_gate, w_out):
    q_dec, k_dec, diag, blk = _decays_np()
    f16 = lambda a: np.ascontiguousarray(a, dtype=np.float16)
    f32 = lambda a: np.ascontiguousarray(a, dtype=np.float32)

    w_qkv_r = np.asarray(w_qkv).reshape(HID, H, 3, D)
    # hidT_p[b]: [8 pr, 128 p, 16 k, 512 t]
    hidT_all = [
        f16(np.asarray(hidden_states[b]).reshape(8, 512, NKT, 128)
            .transpose(0, 3, 2, 1))
        for b in range(BATCH)
    ]
    # hidTq per core: [128 p, 16 k, 1024 lt], lt = b2*512 + mm*128 + t
    hs_r = np.asarray(hidden_states).reshape(BATCH, 32, 128, NKT, 128)
    w_gate_p = f16(np.asarray(w_gate).reshape(NKT, 128, NKT, 128)
                   .transpose(2, 1, 0, 3))
    w_out_p = f16(np.asarray(w_out).reshape(NKT, 128, HID).transpose(1, 0, 2))
    maps = []
    for c in range(N_CORES):
        b, g = divmod(c, HG)
        hs = slice(g * HL, (g + 1) * HL)
        # my half-blocks: 8m + c of each batch; hs_r[b2][hb] is
        # [4 mm, 128 t, 16 k, 128 p] -> [128 p, 16 k, (b2 mm t)]
        hb = [8 * mm + c for mm in range(NCH)]
        hq = np.stack([hs_r[b2][hb] for b2 in range(BATCH)])  # [2,4,128,16,128]
        hq = hq.transpose(4, 3, 0, 1, 2).reshape(128, NKT, TQ)
        wq = np.concatenate(
            [np.ascontiguousarray(w_qkv_r[:, hs, 0, :]).reshape(HID, HL * D),
             np.ascontiguousarray(w_qkv_r[:, hs, 1, :]).reshape(HID, HL * D)],
            axis=1).reshape(NKT, 128, HL * 2 * D).transpose(1, 0, 2)
        wv = (np.ascontiguousarray(w_qkv_r[:, hs, 2, :])
              .reshape(NKT, 128, HL * D).transpose(1, 0, 2))
        maps.append({
            "hidT": hidT_all[b],
            "hidTq": f16(hq),
            "w_qk": f16(wq),
            "w_v": f16(wv),
            "w_gate": w_gate_p,
            "w_out": w_out_p,
            "normw": f32(norm_weight),
            "ddt": f16(diag[hs].transpose(0, 2, 1).reshape(HL, 2, 128, B)),
            "qdbc": _qd_pair(q_dec[hs]),
            "kdc": _qd_pair(k_dec[hs]),
            "bdi": f16(np.eye(D)[None] * blk[hs][:, None, None]),
        })
    return maps


def _gather(res):
    """res[c]["y"] rows are lt = b2*512 + mm*128 + t; global token
    (b2, (8*mm + c)*128 + t)."""
    out = np.empty((BATCH, SEQ, HID), dtype=np.float32)
    for c in range(N_CORES):
        yv = np.asarray(res[c]["y"]).reshape(BATCH, NCH, 128, HID)
        for mm in range(NCH):
            hb = 8 * mm + c
            out[:, hb * 128:(hb + 1) * 128, :] = yv[:, mm]
    return out


def kernel(hidden_states, w_qkv, norm_weight, w_gate, w_out):
    global _cached_nc
    if _cached_nc is None:
        _cached_nc = _build()
    nc = _cached_nc

    maps = _in_maps(hidden_states, w_qkv, norm_weight, w_gate, w_out)
    res = run_bass_kernel_spmd(nc, maps, list(range(N_CORES)))
    return _gather(res.results)
